# revision 33
# baseline (speedup 1.0000x reference)
"""Trainium2 Bass kernel for a binarized (XNOR-style) ResNet BasicBlock.

Reference semantics (per nn_BasicBlock_37228776522124):
    out = BN2(conv3x3(sign(BN1(conv3x3(sign(x), sign(w1)*a1))), sign(w2)*a2)) + x
with training-mode BN (batch stats over N,H,W) and per-out-channel
weight scale a_l = mean(|w_l|).

Key facts exploited:
  * conv inputs are exactly +-1 -> fp8 DoubleRow matmuls accumulate EXACT
    integers in fp32 PSUM (|z| <= 2304 < 2^24).
  * conv(sign(x), sign(w)*a) = a * conv(sign(x), sign(w)); a and BN fold
    into one per-channel affine s*z + b applied post-conv.
  * z is always even; z1 only feeds sign(z - mean) so it is stored as fp8
    at z/16 (sign-safe); z2 is stored fp16 at z/2 (exact).
  * Data-parallel over batch (4 images/core on 8 cores). BN batch stats
    are AllReduced in TWO slices per conv: images 0-2 (hidden under
    image 3's conv) and image 3 alone (short critical-path AR).
  * x is kept resident in SBUF as fp16 for the residual; output is
    written fp16 and converted to fp32 on the host (|err| ~1e-3 << tol).
  * Junk accumulating matmul chains warm the PE HAM clock-gate before
    conv1 and during the AR1b wait so conv streams run at 2.4 GHz.

Self-contained: only needs /opt/trn_rl_repo (the Bass toolchain) + numpy.
"""

import os
import sys

for _p in ("/opt/trn_rl_repo",):
    if os.path.isdir(_p) and _p not in sys.path:
        sys.path.insert(0, _p)

import numpy as np

# Problem shapes (hardcoded per spec)
N_FULL, C, H, W = 32, 256, 56, 56
NCORES = 8
NPER = N_FULL // NCORES          # 4 images per core
SP = H * W                       # 3136
SPH = SP // 2                    # 1568 (28 rows), x is loaded in halves
HP = H + 2                       # 58 (zero-padded)
SPP = HP * HP                    # 3364
NIB = C // 128                   # 2 input-channel blocks
NOB = C // 128                   # 2 output-channel blocks
NTAP = 9
NK = NTAP * NIB                  # 18 accumulation steps per output tile
RB = 7                           # row-blocks of 8 rows
RBW = 8 * W                      # 448 valid outputs per row-block
NMOV = 8 * HP                    # 464 moving columns (8 contiguous pad rows)
RBQ = NMOV + 2                   # 466 f32 <= one psum bank; tap tw writes a
                                 # contiguous 464 window at offset 2-tw; all
                                 # taps agree on q = r*58 + w + 2 for valid w
EPS = 1e-5
KELEM = C * NTAP                 # 2304 weight elems per out channel

_nc_cache = {}


def build_nc(num_devices=NCORES):
    import concourse.bacc as bacc
    import concourse.tile as tile
    import concourse.mybir as mybir
    from concourse.masks import make_identity

    F32 = mybir.dt.float32
    F16 = mybir.dt.float16
    BF16 = mybir.dt.bfloat16
    ALU = mybir.AluOpType
    ACTF = mybir.ActivationFunctionType
    AX = mybir.AxisListType

    nc = bacc.Bacc(
        "TRN2", target_bir_lowering=False, debug=False,
        num_devices=num_devices,
    )

    x_t = nc.dram_tensor("x", [NPER, C, H, W], F32, kind="ExternalInput")
    w_t = [
        nc.dram_tensor("w1", [C, C, 3, 3], F32, kind="ExternalInput"),
        nc.dram_tensor("w2", [C, C, 3, 3], F32, kind="ExternalInput"),
    ]
    g_t = [
        nc.dram_tensor("gamma1", [C], F32, kind="ExternalInput"),
        nc.dram_tensor("gamma2", [C], F32, kind="ExternalInput"),
    ]
    b_t = [
        nc.dram_tensor("beta1", [C], F32, kind="ExternalInput"),
        nc.dram_tensor("beta2", [C], F32, kind="ExternalInput"),
    ]
    # fp16 output: host converts back to fp32 (halves the tail DMA)
    out_t = nc.dram_tensor("out", [NPER, C, H, W], F16, kind="ExternalOutput")

    x_ap = x_t.ap().rearrange("n c h w -> n c (h w)")      # [4, 256, 3136]
    out_ap = out_t.ap().rearrange("n c h w -> n c (h w)")
    rgroups = [list(range(num_devices))]
    M_TOTAL = float(num_devices * NPER * SP)
    # debug bisection: W < C1 < AR1 < C2 < FULL
    phase_lim = {"W": 0, "C1": 1, "AR1": 2, "C2": 3, "FULL": 9}[
        os.environ.get("KERNEL_PHASES", "FULL")]
    no_mm = bool(os.environ.get("KERNEL_NO_MM"))
    # fp8e4m3 DoubleRow matmuls: +-1 exact in fp8, 2 K-rows/cell -> ~2x PE
    use_fp8 = os.environ.get("KERNEL_FP8", "1") == "1"
    A8 = mybir.dt.float8e4
    PM = mybir.MatmulPerfMode
    # abuf block pitch: 2-col left margin (first-tap 466-wide matmul reads
    # from grid-2) + 3364 grid + tail pad; 3376 keeps fp8 pair-step 16B-aligned
    ABW = 3376
    GB = 2                          # grid base offset inside each block
    ABD = A8 if use_fp8 else BF16

    with tile.TileContext(nc) as tc:
        with (
            tc.tile_pool(name="consts", bufs=1) as pc,
            tc.tile_pool(name="dbl", bufs=2) as pd,
            tc.tile_pool(name="psum", bufs=8, space="PSUM") as pp,
            tc.tile_pool(name="dram", bufs=1, space="DRAM") as pdram,
        ):
            ident = pc.tile([128, 128], F32, name="ident", tag="ident")
            make_identity(nc, ident[:])
            epsap = pc.tile([128, 1], F32, name="epsap", tag="epsap")
            nc.vector.memset(epsap[:], EPS)
            # junk fp8 operands for PE HAM warm-up matmul chains
            jw = pc.tile([128, 128], A8, name="jw", tag="jw")
            nc.vector.memset(jw[:], 1.0)
            jwb = pc.tile([128, 4], A8, name="jwb", tag="jwb")

            # persistent stores: z1 fp8 at z/16 (feeds sign only), z2 fp16
            # at z/2 (exact, feeds the output affine)
            zstore = [
                pc.tile([128, NPER * NOB * SP], A8 if l == 0 else F16,
                        name=f"z{l}", tag=f"z{l}")
                for l in range(2)
            ]
            zscale = [1.0 / 16.0, 0.5]
            # x resident in SBUF as fp16 for the residual add
            x16 = pc.tile([128, NPER * NOB * SP], F16, name="x16", tag="x16")
            # two persistent activation buffers, alternated per image; the
            # zero pad ring + margins are written ONCE here (the interior is
            # fully overwritten by each image's sign fill), removing all
            # per-image memsets and their queue-ordering entanglement.
            ab2 = [pc.tile([128, NIB * ABW], ABD, name=f"ab{i}", tag=f"ab{i}")
                   for i in range(2)]
            for i in range(2):
                nc.vector.memset(ab2[i][:], 0.0)
            wsign = [
                pc.tile([128, NK * NOB * 128], ABD, name=f"ws{l}", tag=f"ws{l}")
                for l in range(2)
            ]
            alphar = [pc.tile([128, NOB], F32, name=f"al{l}", tag=f"al{l}") for l in range(2)]
            sumc = [pc.tile([128, NOB * 28], F32, name=f"sc{l}", tag=f"sc{l}") for l in range(2)]
            sqc = [pc.tile([128, NOB * 28], F32, name=f"qc{l}", tag=f"qc{l}") for l in range(2)]
            # split batch stats: part 0 = images 0-2, part 1 = image 3
            statp = [[pc.tile([128, 4], F32, name=f"sl{l}{p}", tag=f"sl{l}{p}")
                      for p in range(2)] for l in range(2)]
            statg2 = [[pc.tile([128, 4], F32, name=f"sg{l}{p}", tag=f"sg{l}{p}")
                       for p in range(2)] for l in range(2)]
            statg = [pc.tile([128, 4], F32, name=f"st{l}", tag=f"st{l}") for l in range(2)]
            gb = [pc.tile([128, 2 * NOB], F32, name=f"gb{l}", tag=f"gb{l}") for l in range(2)]
            coef = [pc.tile([128, 2 * NOB], F32, name=f"cf{l}", tag=f"cf{l}") for l in range(2)]
            btmp = [pc.tile([128, 16], F32, name=f"bt{l}", tag=f"bt{l}") for l in range(2)]

            # dummy AllReduce at kernel start: absorbs the first-collective
            # latency (~60us) concurrently with conv1 so the real ARs are fast
            ard_i = pdram.tile([128, 1], F32, name="ard_i", tag="ard_i")
            ard_o = pdram.tile([128, 1], F32, name="ard_o", tag="ard_o")
            nc.gpsimd.dma_start(ard_i[:], g_t[0].ap()[0:128])
            nc.gpsimd.collective_compute(
                "AllReduce", ALU.add, replica_groups=rgroups,
                ins=[ard_i.opt()], outs=[ard_o.opt()],
            )
            # (the DCE-keeping park of ard_o is emitted after conv1 so its
            # ~90us wait doesn't head-block a queue anything rides on)

            # ---------------- weight prep ----------------
            # w/gamma/beta DMAs ride the scalar queue so the sync queue is
            # dedicated to x loads (image 0's x gates the first conv matmul).
            def weight_prep(l):
                wd = w_t[l].ap().rearrange("o i h w -> o (i h w)")  # [256,2304]
                KH = KELEM // NIB                       # 1152 per i-block
                for ob in range(NOB):
                    for ib in range(NIB):
                        # load per i-block halves: smaller SBUF footprint and
                        # finer WAR pacing against the next layer's loads
                        wraw = pc.tile([128, KH], F32, name="wraw",
                                       tag="wraw", bufs=2)
                        nc.scalar.dma_start(
                            wraw[:],
                            wd[ob * 128:(ob + 1) * 128, ib * KH:(ib + 1) * KH])
                        # alpha_raw = sum |w|, accumulated over both halves
                        acol = (alphar[l][:, ob:ob + 1] if ib == 0
                                else btmp[l][:, 14:15])
                        nc.vector.tensor_reduce(
                            out=acol, in_=wraw[:],
                            axis=AX.X, op=ALU.add, apply_absolute_value=True,
                        )
                        if ib == 1:
                            nc.vector.tensor_add(
                                alphar[l][:, ob:ob + 1],
                                alphar[l][:, ob:ob + 1], acol)
                        wtap = wraw[:].rearrange("p (i t) -> p t i", t=NTAP)
                        for t in range(NTAP):
                            if use_fp8:
                                # [p=i, (ob,t,pair=ib), m=o] for DoubleRow
                                kidx = (ob * NTAP + t) * 2 + ib
                            else:
                                kidx = ob * NK + t * NIB + ib
                            psT = pp.tile([128, RBW], F32, name="cps", tag="cps")
                            # transpose [o,i] -> [i,o] through the PE
                            nc.tensor.transpose(
                                psT[:, 0:128],
                                wtap[:, t, :],
                                ident[:],
                            )
                            nc.scalar.activation(
                                out=wsign[l][:, kidx * 128:(kidx + 1) * 128],
                                in_=psT[:, 0:128], func=ACTF.Sign,
                            )
                # gamma/beta -> [128, col]
                for ob in range(NOB):
                    nc.scalar.dma_start(
                        gb[l][:, ob:ob + 1],
                        g_t[l].ap()[ob * 128:(ob + 1) * 128],
                    )
                    nc.scalar.dma_start(
                        gb[l][:, NOB + ob:NOB + ob + 1],
                        b_t[l].ap()[ob * 128:(ob + 1) * 128],
                    )
                # off-critical fold precomputes: alp = alpha_raw/KELEM,
                # aa = alp^2, agz = gamma*alp/zscale
                pre = btmp[l]
                nc.vector.tensor_scalar_mul(
                    pre[:, 6:8], alphar[l][:, 0:2], 1.0 / KELEM)
                nc.vector.tensor_mul(pre[:, 0:2], pre[:, 6:8], pre[:, 6:8])
                nc.vector.tensor_mul(pre[:, 2:4], pre[:, 6:8], gb[l][:, 0:2])
                nc.vector.tensor_scalar_mul(
                    pre[:, 2:4], pre[:, 2:4], 1.0 / zscale[l])

            weight_prep(0)
            jw1b = pc.tile([128, 4], A8, name="jw1b", tag="jw1b")

            # ---------------- stats AllReduce (split) ----------------
            arin = [[pdram.tile([128, 4], F32, name=f"ari{l}{p}", tag=f"ari{l}{p}")
                     for p in range(2)] for l in range(2)]
            arout = [[pdram.tile([128, 4], F32, name=f"aro{l}{p}", tag=f"aro{l}{p}")
                      for p in range(2)] for l in range(2)]

            def issue_stats_ar(l, part):
                # part 0: images 0-2 (cols 0:21) -- hidden under image 3's
                # conv; part 1: image 3 (cols 21:28) -- short critical AR.
                lo, hi = (0, 21) if part == 0 else (21, 28)
                st = statp[l][part]
                for ob in range(NOB):
                    nc.vector.tensor_reduce(
                        out=st[:, ob:ob + 1],
                        in_=sumc[l][:, ob * 28 + lo:ob * 28 + hi],
                        axis=AX.X, op=ALU.add,
                    )
                    nc.vector.tensor_reduce(
                        out=st[:, NOB + ob:NOB + ob + 1],
                        in_=sqc[l][:, ob * 28 + lo:ob * 28 + hi],
                        axis=AX.X, op=ALU.add,
                    )
                # AR-side DMAs ride the sync HWDGE queue (lower latency);
                # emission order keeps every wait behind nothing that is
                # needed earlier than the wait resolves.
                nc.sync.dma_start(arin[l][part][:], st[:])
                nc.gpsimd.collective_compute(
                    "AllReduce", ALU.add, replica_groups=rgroups,
                    ins=[arin[l][part].opt()], outs=[arout[l][part].opt()],
                )
                nc.sync.dma_start(statg2[l][part][:], arout[l][part][:])

            def fold_bn(l):
                # statg = statA + statB; then with sm = Ssum/M (z'-units,
                # sm = zs*mean) and e2 = Ssq/M:
                #   var = e2 - sm^2/zs^2 ; inv = rsqrt(aa*var + eps)
                #   coef_s = agz*inv ; coef_b = beta - coef_s*sm
                # (aa, agz precomputed off-critical in weight_prep)
                pre = btmp[l]
                nc.vector.tensor_add(
                    statg[l][:], statg2[l][0][:], statg2[l][1][:])
                nc.vector.tensor_scalar_mul(
                    pre[:, 4:8], statg[l][:, 0:4], 1.0 / M_TOTAL)
                w = pre[:, 8:10]
                nc.vector.tensor_mul(w, pre[:, 4:6], pre[:, 4:6])
                nc.vector.tensor_scalar_mul(
                    w, w, 1.0 / (zscale[l] * zscale[l]))
                nc.vector.tensor_sub(w, pre[:, 6:8], w)           # var
                nc.vector.tensor_mul(w, w, pre[:, 0:2])           # aa*var
                nc.scalar.activation(pre[:, 10:12], w, ACTF.Sqrt, bias=epsap[:])
                nc.vector.reciprocal(w, pre[:, 10:12])            # inv
                nc.vector.tensor_mul(coef[l][:, 0:2], pre[:, 2:4], w)
                nc.vector.tensor_mul(w, coef[l][:, 0:2], pre[:, 4:6])
                nc.vector.tensor_sub(coef[l][:, 2:4], gb[l][:, 2:4], w)
                if l == 0:
                    # thr = -coef_b/coef_s for fill2's DVE-emulated sign
                    # (gamma > 0 so coef_s > 0 and sign(s*z+b)=sign(z-thr))
                    thr = btmp[0][:, 14:16]
                    nc.vector.reciprocal(thr, coef[0][:, 0:2])
                    nc.vector.tensor_mul(thr, thr, coef[0][:, 2:4])
                    nc.vector.tensor_scalar_mul(thr, thr, -1.0)

            # ---------------- one conv pass (shared for conv1/conv2) --------
            def conv_pass(l, act_fill, do_ar=True):
                """act_fill(n, abuf) writes signed fp8 acts into the padded
                [128, NIB*SPP] buffer interior (ring already zero).
                Image n+1's fill is emitted BEFORE image n's matmuls/drains
                so on the in-order ACT/DVE/sync queues the next image's
                loads and signs run during the current matmul stream."""
                act_fill(0, ab2[0])
                if l == 0:
                    # PE HAM warm-up gated on image 0's x arrival (jw1b is
                    # written from the 3rd x chunk): the junk-matmul chain
                    # runs right before conv1's first real matmul, so the
                    # clock-gate is warm with no idle window in between.
                    nc.vector.tensor_scalar(
                        out=jw1b[:], in0=xin_handles[(0, 1, 0)][:, 0:4],
                        scalar1=0.0, scalar2=None, op0=ALU.mult)
                    warmps = pp.tile([128, RBQ], F32, name="cps", tag="cps")
                    NWARM1 = 48
                    for i in range(NWARM1):
                        nc.tensor.matmul(warmps[:, 0:128], jw[:], jw[:],
                                         start=(i == 0),
                                         stop=(i == NWARM1 - 1))
                        if i == 0:
                            nc.tensor.matmul(warmps[:, 0:4], jw[:], jw1b[:],
                                             start=False, stop=False)
                    nc.vector.tensor_reduce(
                        out=btmp[0][:, 13:14], in_=warmps[:, 0:128],
                        axis=AX.X, op=ALU.add)
                for n in range(NPER):
                    abuf = ab2[n % 2]
                    if n + 1 < NPER:
                        act_fill(n + 1, ab2[(n + 1) % 2])
                    if n == NPER - 1 and do_ar:
                        # images 0-2 are done draining by now: AllReduce
                        # their stats, hidden under image 3's matmul stream.
                        # Emitted after image 3's fill so its x DMAs aren't
                        # stuck behind this wait on the sync queue.
                        issue_stats_ar(l, 0)
                    for ob in range(NOB):
                        ps = [pp.tile([128, RBQ], F32, name="cps", tag="cps")
                              for _ in range(RB)]
                        if use_fp8:
                            ab3 = abuf[:].rearrange(
                                "p (two s) -> p two s", two=NIB)
                            for t in range(NTAP):
                                th, tw = t // 3, t % 3
                                base = (ob * NTAP + t) * 2 * 128
                                lhsT = wsign[l][:, base:base + 256].rearrange(
                                    "p (two m) -> p two m", two=2)
                                for rb in range(RB):
                                    r0 = (rb * 8 + th) * HP
                                    if t == 0:
                                        # 466-wide: covers the whole psum
                                        # tile so has_written is uniform
                                        rhs = ab3[:, :, r0:r0 + RBQ]
                                        outap = ps[rb][:, 0:RBQ]
                                    else:
                                        rhs = ab3[:, :, GB + r0:GB + r0 + NMOV]
                                        outap = ps[rb][:, 2 - tw:2 - tw + NMOV]
                                    nc.tensor.matmul(
                                        outap, lhsT, rhs,
                                        start=(t == 0), stop=(t == NTAP - 1),
                                        perf_mode=PM.DoubleRow,
                                    )
                        else:
                            for k in range(NK):
                                t, ib = k // NIB, k % NIB
                                th, tw = t // 3, t % 3
                                kidx = ob * NK + k
                                af = abuf[:, ib * ABW:(ib + 1) * ABW]
                                lhsT = wsign[l][:, kidx * 128:(kidx + 1) * 128]
                                for rb in range(RB):
                                    r0 = (rb * 8 + th) * HP
                                    if no_mm and k > 0:
                                        continue
                                    if k == 0:
                                        rhs = af[:, r0:r0 + RBQ]
                                        outap = ps[rb][:, 0:RBQ]
                                    else:
                                        rhs = af[:, GB + r0:GB + r0 + NMOV]
                                        outap = ps[rb][:, 2 - tw:2 - tw + NMOV]
                                    nc.tensor.matmul(
                                        outap, lhsT, rhs,
                                        start=(k == 0),
                                        stop=(k == NK - 1) or no_mm,
                                    )
                        zs = zstore[l]
                        for rb in range(RB):
                            col = n * RB + rb
                            zsl = zs[:, ((n * NOB + ob) * SP + rb * RBW):
                                      ((n * NOB + ob) * SP + (rb + 1) * RBW)
                                      ].rearrange("p (h w) -> p h w", w=W)
                            qv = ps[rb][:, 2:2 + NMOV].rearrange(
                                "p (h w) -> p h w", w=HP)[:, :, 0:W]
                            # z*zscale store on DVE; accum_out = sum(z*zscale)
                            nc.vector.tensor_scalar(
                                out=zsl, in0=qv,
                                scalar1=zscale[l], scalar2=None, op0=ALU.mult,
                                op1=ALU.add,
                                accum_out=sumc[l][:, ob * 28 + col:
                                                  ob * 28 + col + 1],
                            )
                            # scr = z^2 (dummy out); accum = sum(z^2)
                            scr = pd.tile([128, RBW], F16, name="scr", tag="scr")
                            nc.scalar.activation(
                                out=scr[:].rearrange("p (h w) -> p h w", w=W),
                                in_=qv, func=ACTF.Square,
                                accum_out=sqc[l][:, ob * 28 + col:
                                                 ob * 28 + col + 1],
                            )
                if not do_ar:
                    return
                issue_stats_ar(l, 1)
                fold_bn(l)

            # ---------------- conv1: acts = sign(x) ----------------
            # x arrives in fp32 halves on the sync queue; each half is
            # sign-ed into abuf (exact, from fp32) and copied to the
            # resident fp16 x16 on the gpsimd engine (idle during conv).
            xin_handles = {}

            def fill1(n, abuf):
                for ib in range(NIB):
                    a58 = abuf[:, ib * ABW + GB:ib * ABW + GB + SPP].rearrange(
                        "p (h w) -> p h w", w=HP)
                    xsl = x16[:, (n * NOB + ib) * SP:(n * NOB + ib + 1) * SP]
                    for h in range(2):
                        xin = pd.tile([128, SPH], F32, name="xin", tag="xin",
                                      bufs=3)
                        xin_handles[(n, ib, h)] = xin
                        nc.sync.dma_start(
                            xin[:],
                            x_ap[n, ib * 128:(ib + 1) * 128,
                                 h * SPH:(h + 1) * SPH])
                        xv = xin[:].rearrange("p (h w) -> p h w", w=W)
                        # image 0's fill is on the critical path (runs at
                        # high priority, ahead of the weight-prep signs);
                        # later images' prefetched fills are demoted so the
                        # current image's psum drains win the engine.
                        with tc.high_priority(offset=-3000 if n else None):
                            nc.vector.tensor_copy(
                                xsl[:, h * SPH:(h + 1) * SPH], xin[:])
                            nc.scalar.activation(
                                out=a58[:, 1 + h * 28:1 + (h + 1) * 28,
                                        1:W + 1],
                                in_=xv, func=ACTF.Sign)

            if phase_lim >= 1:
                conv_pass(0, fill1, do_ar=(phase_lim >= 2))

            # park the dummy-AR result so DCE keeps it; the gpsimd queue
            # already waits for the collective, so the park adds no blocking
            # there (on any other queue the scheduler may slot it mid-stream
            # and stall that queue until the dummy AR completes).
            nc.gpsimd.dma_start(btmp[0][:, 12:13], ard_o[:])

            # conv2 weight prep here: its PE transposes overlap the AR1 wait
            weight_prep(1)

            if phase_lim >= 2:
                # PE HAM warm-up: the chain's second link reads jwb (a DVE
                # write gated on the fold output), so on the in-order PE
                # queue the chain runs right after AR1b's fold and keeps
                # the PE busy through fill2 -- conv2 then starts warm.
                nc.vector.tensor_scalar(
                    out=jwb[:], in0=coef[0][:, 0:4],
                    scalar1=0.0, scalar2=None, op0=ALU.mult)
                warm2 = pp.tile([128, RBQ], F32, name="cps", tag="cps")
                NWARM2 = 96
                for i in range(NWARM2):
                    nc.tensor.matmul(warm2[:, 0:128], jw[:], jw[:],
                                     start=(i == 0), stop=(i == NWARM2 - 1))
                    if i == 0:
                        nc.tensor.matmul(warm2[:, 0:4], jw[:], jwb[:],
                                         start=False, stop=False)
                nc.vector.tensor_reduce(
                    out=btmp[1][:, 13:14], in_=warm2[:, 0:128],
                    axis=AX.X, op=ALU.add)

            # ---------------- conv2: acts = sign(s1*z1 + b1) ----------------
            def fill2(n, abuf):
                for ib in range(NIB):
                    a58 = abuf[:, ib * ABW + GB:ib * ABW + GB + SPP].rearrange(
                        "p (h w) -> p h w", w=HP)
                    zv = zstore[0][:, (n * NOB + ib) * SP:
                                   (n * NOB + ib + 1) * SP].rearrange(
                        "p (h w) -> p h w", w=W)
                    with tc.high_priority(offset=-3000 if n else None):
                        if ib == 0:
                            nc.scalar.activation(
                                out=a58[:, 1:H + 1, 1:W + 1], in_=zv,
                                func=ACTF.Sign,
                                scale=coef[0][:, ib:ib + 1],
                                bias=coef[0][:, NOB + ib:NOB + ib + 1],
                            )
                        else:
                            # DVE-emulated sign so the two channel blocks
                            # fill in parallel on ACT and DVE:
                            # (z >= thr)*2 - 1 = sign(s*z + b) for s > 0
                            av = a58[:, 1:H + 1, 1:W + 1]
                            nc.vector.tensor_scalar(
                                out=av, in0=zv,
                                scalar1=btmp[0][:, 14 + ib:15 + ib],
                                scalar2=2.0, op0=ALU.is_ge, op1=ALU.mult)
                            nc.vector.tensor_scalar_add(av, av, -1.0)

            if phase_lim >= 3:
                conv_pass(1, fill2, do_ar=(phase_lim >= 9))

            if phase_lim < 9:
                # debug: dump something touching live tiles into out
                dbg = pd.tile([128, SP], F16, name="dbg", tag="t16", bufs=4)
                if phase_lim >= 1:
                    nc.vector.tensor_copy(dbg[:], zstore[0][:, 0:SP])
                else:
                    nc.vector.tensor_copy(dbg[:], wsign[0][:, 0:SP])
                nc.sync.dma_start(out_ap[0, 0:128, :], dbg[:])

            # ---------------- finalize: out = s2*z2' + b2 + x ----------------
            # Per block: per-channel affine (alternating ACT / DVE so neither
            # engine is the serial bottleneck), DVE residual add from the
            # resident fp16 x, fp16 store rotated over 3 DMA queues.
            for n in range(NPER if phase_lim >= 9 else 0):
                for ob in range(NOB):
                    k = n * NOB + ob
                    zsl = zstore[1][:, (n * NOB + ob) * SP:
                                    (n * NOB + ob + 1) * SP]
                    xsl = x16[:, (n * NOB + ob) * SP:(n * NOB + ob + 1) * SP]
                    t16 = pd.tile([128, SP], F16, name="t16", tag="t16",
                                  bufs=4)
                    if k % 2 == 0 or k == 7:
                        nc.scalar.activation(
                            out=t16[:], in_=zsl, func=ACTF.Identity,
                            scale=coef[1][:, ob:ob + 1],
                            bias=coef[1][:, NOB + ob:NOB + ob + 1],
                        )
                    else:
                        nc.vector.tensor_scalar(
                            out=t16[:], in0=zsl,
                            scalar1=coef[1][:, ob:ob + 1],
                            scalar2=coef[1][:, NOB + ob:NOB + ob + 1],
                            op0=ALU.mult, op1=ALU.add,
                        )
                    nc.vector.tensor_add(t16[:], t16[:], xsl)
                    dma_eng = (nc.sync, nc.scalar)[k % 2]
                    dma_eng.dma_start(
                        out_ap[n, ob * 128:(ob + 1) * 128, :], t16[:])

    nc.compile()
    return nc


def _get_nc(num_devices=NCORES):
    if num_devices not in _nc_cache:
        _nc_cache[num_devices] = build_nc(num_devices)
    return _nc_cache[num_devices]


def kernel(**inputs):
    from concourse.bass_utils import run_bass_kernel_spmd

    nc = _get_nc(NCORES)
    x = np.ascontiguousarray(np.asarray(inputs["x"], dtype=np.float32))
    shared = {
        k: np.ascontiguousarray(np.asarray(inputs[k], dtype=np.float32))
        for k in ("w1", "gamma1", "beta1", "w2", "gamma2", "beta2")
    }
    in_maps = [
        {"x": x[c * NPER:(c + 1) * NPER], **shared} for c in range(NCORES)
    ]
    res = run_bass_kernel_spmd(nc, in_maps, core_ids=list(range(NCORES)))
    out = np.concatenate([np.asarray(r["out"]) for r in res.results], axis=0)
    return out.astype(np.float32)


# revision 34
# speedup vs baseline: 1.0418x; 1.0418x over previous
"""Trainium2 Bass kernel for a binarized (XNOR-style) ResNet BasicBlock.

Reference semantics (per nn_BasicBlock_37228776522124):
    out = BN2(conv3x3(sign(BN1(conv3x3(sign(x), sign(w1)*a1))), sign(w2)*a2)) + x
with training-mode BN (batch stats over N,H,W) and per-out-channel
weight scale a_l = mean(|w_l|).

Key facts exploited:
  * conv inputs are exactly +-1 -> fp8 DoubleRow matmuls accumulate EXACT
    integers in fp32 PSUM (|z| <= 2304 < 2^24).
  * conv(sign(x), sign(w)*a) = a * conv(sign(x), sign(w)); a and BN fold
    into one per-channel affine s*z + b applied post-conv.
  * z is always even; z1 only feeds sign(z - mean) so it is stored as fp8
    at z/16 (sign-safe); z2 is stored fp16 at z/2 (exact).
  * Data-parallel over batch (4 images/core on 8 cores). BN batch stats
    are AllReduced in TWO slices per conv: images 0-2 (hidden under
    image 3's conv) and image 3 alone (short critical-path AR).
  * x is kept resident in SBUF as fp16 for the residual; output is
    written fp16 and converted to fp32 on the host (|err| ~1e-3 << tol).
  * Junk accumulating matmul chains warm the PE HAM clock-gate before
    conv1 and during the AR1b wait so conv streams run at 2.4 GHz.

Self-contained: only needs /opt/trn_rl_repo (the Bass toolchain) + numpy.
"""

import os
import sys

for _p in ("/opt/trn_rl_repo",):
    if os.path.isdir(_p) and _p not in sys.path:
        sys.path.insert(0, _p)

import numpy as np

# Problem shapes (hardcoded per spec)
N_FULL, C, H, W = 32, 256, 56, 56
NCORES = 8
NPER = N_FULL // NCORES          # 4 images per core
SP = H * W                       # 3136
SPH = SP // 2                    # 1568 (28 rows), x is loaded in halves
HP = H + 2                       # 58 (zero-padded)
SPP = HP * HP                    # 3364
NIB = C // 128                   # 2 input-channel blocks
NOB = C // 128                   # 2 output-channel blocks
NTAP = 9
NK = NTAP * NIB                  # 18 accumulation steps per output tile
RB = 7                           # row-blocks of 8 rows
RBW = 8 * W                      # 448 valid outputs per row-block
NMOV = 8 * HP                    # 464 moving columns (8 contiguous pad rows)
RBQ = NMOV + 2                   # 466 f32 <= one psum bank; tap tw writes a
                                 # contiguous 464 window at offset 2-tw; all
                                 # taps agree on q = r*58 + w + 2 for valid w
EPS = 1e-5
KELEM = C * NTAP                 # 2304 weight elems per out channel

_nc_cache = {}


def build_nc(num_devices=NCORES):
    import concourse.bacc as bacc
    import concourse.tile as tile
    import concourse.mybir as mybir
    from concourse.masks import make_identity

    F32 = mybir.dt.float32
    F16 = mybir.dt.float16
    BF16 = mybir.dt.bfloat16
    ALU = mybir.AluOpType
    ACTF = mybir.ActivationFunctionType
    AX = mybir.AxisListType

    nc = bacc.Bacc(
        "TRN2", target_bir_lowering=False, debug=False,
        num_devices=num_devices,
    )

    x_t = nc.dram_tensor("x", [NPER, C, H, W], F32, kind="ExternalInput")
    w_t = [
        nc.dram_tensor("w1", [C, C, 3, 3], F32, kind="ExternalInput"),
        nc.dram_tensor("w2", [C, C, 3, 3], F32, kind="ExternalInput"),
    ]
    g_t = [
        nc.dram_tensor("gamma1", [C], F32, kind="ExternalInput"),
        nc.dram_tensor("gamma2", [C], F32, kind="ExternalInput"),
    ]
    b_t = [
        nc.dram_tensor("beta1", [C], F32, kind="ExternalInput"),
        nc.dram_tensor("beta2", [C], F32, kind="ExternalInput"),
    ]
    # fp16 output: host converts back to fp32 (halves the tail DMA)
    out_t = nc.dram_tensor("out", [NPER, C, H, W], F16, kind="ExternalOutput")

    x_ap = x_t.ap().rearrange("n c h w -> n c (h w)")      # [4, 256, 3136]
    out_ap = out_t.ap().rearrange("n c h w -> n c (h w)")
    rgroups = [list(range(num_devices))]
    M_TOTAL = float(num_devices * NPER * SP)
    # debug bisection: W < C1 < AR1 < C2 < FULL
    phase_lim = {"W": 0, "C1": 1, "AR1": 2, "C2": 3, "FULL": 9}[
        os.environ.get("KERNEL_PHASES", "FULL")]
    no_mm = bool(os.environ.get("KERNEL_NO_MM"))
    # fp8e4m3 DoubleRow matmuls: +-1 exact in fp8, 2 K-rows/cell -> ~2x PE
    use_fp8 = os.environ.get("KERNEL_FP8", "1") == "1"
    A8 = mybir.dt.float8e4
    PM = mybir.MatmulPerfMode
    # abuf block pitch: 2-col left margin (first-tap 466-wide matmul reads
    # from grid-2) + 3364 grid + tail pad; 3376 keeps fp8 pair-step 16B-aligned
    ABW = 3376
    GB = 2                          # grid base offset inside each block
    ABD = A8 if use_fp8 else BF16

    with tile.TileContext(nc) as tc:
        with (
            tc.tile_pool(name="consts", bufs=1) as pc,
            tc.tile_pool(name="dbl", bufs=2) as pd,
            tc.tile_pool(name="psum", bufs=8, space="PSUM") as pp,
            tc.tile_pool(name="dram", bufs=1, space="DRAM") as pdram,
        ):
            ident = pc.tile([128, 128], F32, name="ident", tag="ident")
            make_identity(nc, ident[:])
            epsap = pc.tile([128, 1], F32, name="epsap", tag="epsap")
            nc.vector.memset(epsap[:], EPS)
            # junk fp8 operands for PE HAM warm-up matmul chains
            jw = pc.tile([128, 128], A8, name="jw", tag="jw")
            nc.vector.memset(jw[:], 1.0)
            jwb = pc.tile([128, 4], A8, name="jwb", tag="jwb")

            # persistent stores: z1 fp8 at z/16 (feeds sign only), z2 fp16
            # at z/2 (exact, feeds the output affine)
            zstore = [
                pc.tile([128, NPER * NOB * SP], A8 if l == 0 else F16,
                        name=f"z{l}", tag=f"z{l}")
                for l in range(2)
            ]
            zscale = [1.0 / 16.0, 0.5]
            # x resident in SBUF as fp16 for the residual add
            x16 = pc.tile([128, NPER * NOB * SP], F16, name="x16", tag="x16")
            # two persistent activation buffers, alternated per image; the
            # zero pad ring + margins are written ONCE here (the interior is
            # fully overwritten by each image's sign fill), removing all
            # per-image memsets and their queue-ordering entanglement.
            ab2 = [pc.tile([128, NIB * ABW], ABD, name=f"ab{i}", tag=f"ab{i}")
                   for i in range(2)]
            for i in range(2):
                nc.vector.memset(ab2[i][:], 0.0)
            wsign = [
                pc.tile([128, NK * NOB * 128], ABD, name=f"ws{l}", tag=f"ws{l}")
                for l in range(2)
            ]
            alphar = [pc.tile([128, NOB], F32, name=f"al{l}", tag=f"al{l}") for l in range(2)]
            sumc = [pc.tile([128, NOB * 28], F32, name=f"sc{l}", tag=f"sc{l}") for l in range(2)]
            sqc = [pc.tile([128, NOB * 28], F32, name=f"qc{l}", tag=f"qc{l}") for l in range(2)]
            # split batch stats: part 0 = images 0-2, part 1 = image 3
            statp = [[pc.tile([128, 4], F32, name=f"sl{l}{p}", tag=f"sl{l}{p}")
                      for p in range(2)] for l in range(2)]
            statg2 = [[pc.tile([128, 4], F32, name=f"sg{l}{p}", tag=f"sg{l}{p}")
                       for p in range(2)] for l in range(2)]
            statg = [pc.tile([128, 4], F32, name=f"st{l}", tag=f"st{l}") for l in range(2)]
            gb = [pc.tile([128, 2 * NOB], F32, name=f"gb{l}", tag=f"gb{l}") for l in range(2)]
            coef = [pc.tile([128, 2 * NOB], F32, name=f"cf{l}", tag=f"cf{l}") for l in range(2)]
            btmp = [pc.tile([128, 16], F32, name=f"bt{l}", tag=f"bt{l}") for l in range(2)]

            # dummy AllReduce at kernel start: absorbs the first-collective
            # latency (~60us) concurrently with conv1 so the real ARs are fast
            ard_i = pdram.tile([128, 1], F32, name="ard_i", tag="ard_i")
            ard_o = pdram.tile([128, 1], F32, name="ard_o", tag="ard_o")
            nc.gpsimd.dma_start(ard_i[:], g_t[0].ap()[0:128])
            nc.gpsimd.collective_compute(
                "AllReduce", ALU.add, replica_groups=rgroups,
                ins=[ard_i.opt()], outs=[ard_o.opt()],
            )
            # (the DCE-keeping park of ard_o is emitted after conv1 so its
            # ~90us wait doesn't head-block a queue anything rides on)

            # ---------------- weight prep ----------------
            # w/gamma/beta DMAs ride the scalar queue so the sync queue is
            # dedicated to x loads (image 0's x gates the first conv matmul).
            def weight_prep(l):
                wd = w_t[l].ap().rearrange("o i h w -> o (i h w)")  # [256,2304]
                KH = KELEM // NIB                       # 1152 per i-block
                for ob in range(NOB):
                    for ib in range(NIB):
                        # load per i-block halves: smaller SBUF footprint and
                        # finer WAR pacing against the next layer's loads
                        wraw = pc.tile([128, KH], F32, name="wraw",
                                       tag="wraw", bufs=2)
                        nc.scalar.dma_start(
                            wraw[:],
                            wd[ob * 128:(ob + 1) * 128, ib * KH:(ib + 1) * KH])
                        # alpha_raw = sum |w|, accumulated over both halves
                        acol = (alphar[l][:, ob:ob + 1] if ib == 0
                                else btmp[l][:, 14:15])
                        nc.vector.tensor_reduce(
                            out=acol, in_=wraw[:],
                            axis=AX.X, op=ALU.add, apply_absolute_value=True,
                        )
                        if ib == 1:
                            nc.vector.tensor_add(
                                alphar[l][:, ob:ob + 1],
                                alphar[l][:, ob:ob + 1], acol)
                        wtap = wraw[:].rearrange("p (i t) -> p t i", t=NTAP)
                        for t in range(NTAP):
                            if use_fp8:
                                # [p=i, (ob,t,pair=ib), m=o] for DoubleRow
                                kidx = (ob * NTAP + t) * 2 + ib
                            else:
                                kidx = ob * NK + t * NIB + ib
                            psT = pp.tile([128, RBW], F32, name="cps", tag="cps")
                            # transpose [o,i] -> [i,o] through the PE
                            nc.tensor.transpose(
                                psT[:, 0:128],
                                wtap[:, t, :],
                                ident[:],
                            )
                            nc.scalar.activation(
                                out=wsign[l][:, kidx * 128:(kidx + 1) * 128],
                                in_=psT[:, 0:128], func=ACTF.Sign,
                            )
                # gamma/beta -> [128, col]
                for ob in range(NOB):
                    nc.scalar.dma_start(
                        gb[l][:, ob:ob + 1],
                        g_t[l].ap()[ob * 128:(ob + 1) * 128],
                    )
                    nc.scalar.dma_start(
                        gb[l][:, NOB + ob:NOB + ob + 1],
                        b_t[l].ap()[ob * 128:(ob + 1) * 128],
                    )
                # off-critical fold precomputes: alp = alpha_raw/KELEM,
                # aa = alp^2, agz = gamma*alp/zscale
                pre = btmp[l]
                nc.vector.tensor_scalar_mul(
                    pre[:, 6:8], alphar[l][:, 0:2], 1.0 / KELEM)
                nc.vector.tensor_mul(pre[:, 0:2], pre[:, 6:8], pre[:, 6:8])
                nc.vector.tensor_mul(pre[:, 2:4], pre[:, 6:8], gb[l][:, 0:2])
                nc.vector.tensor_scalar_mul(
                    pre[:, 2:4], pre[:, 2:4], 1.0 / zscale[l])

            weight_prep(0)
            jw1b = pc.tile([128, 4], A8, name="jw1b", tag="jw1b")

            # ---------------- stats AllReduce (split) ----------------
            arin = [[pdram.tile([128, 4], F32, name=f"ari{l}{p}", tag=f"ari{l}{p}")
                     for p in range(2)] for l in range(2)]
            arout = [[pdram.tile([128, 4], F32, name=f"aro{l}{p}", tag=f"aro{l}{p}")
                      for p in range(2)] for l in range(2)]

            def issue_stats_ar(l, part):
                # part 0: images 0-2 (cols 0:21) -- hidden under image 3's
                # conv; part 1: image 3 (cols 21:28) -- short critical AR.
                lo, hi = (0, 21) if part == 0 else (21, 28)
                st = statp[l][part]
                for ob in range(NOB):
                    nc.vector.tensor_reduce(
                        out=st[:, ob:ob + 1],
                        in_=sumc[l][:, ob * 28 + lo:ob * 28 + hi],
                        axis=AX.X, op=ALU.add,
                    )
                    nc.vector.tensor_reduce(
                        out=st[:, NOB + ob:NOB + ob + 1],
                        in_=sqc[l][:, ob * 28 + lo:ob * 28 + hi],
                        axis=AX.X, op=ALU.add,
                    )
                # arin DMAs ride the sync HWDGE queue (they only wait on
                # local stats, so they never head-block each other); the
                # DMA-backs ride the gpsimd queue, which collective_compute
                # already blocks until the AR completes -- putting them
                # anywhere else would head-block that queue on the AR.
                nc.sync.dma_start(arin[l][part][:], st[:])
                nc.gpsimd.collective_compute(
                    "AllReduce", ALU.add, replica_groups=rgroups,
                    ins=[arin[l][part].opt()], outs=[arout[l][part].opt()],
                )
                nc.gpsimd.dma_start(statg2[l][part][:], arout[l][part][:])

            def fold_bn(l):
                # statg = statA + statB; then with sm = Ssum/M (z'-units,
                # sm = zs*mean) and e2 = Ssq/M:
                #   var = e2 - sm^2/zs^2 ; inv = rsqrt(aa*var + eps)
                #   coef_s = agz*inv ; coef_b = beta - coef_s*sm
                # (aa, agz precomputed off-critical in weight_prep)
                pre = btmp[l]
                nc.vector.tensor_add(
                    statg[l][:], statg2[l][0][:], statg2[l][1][:])
                nc.vector.tensor_scalar_mul(
                    pre[:, 4:8], statg[l][:, 0:4], 1.0 / M_TOTAL)
                w = pre[:, 8:10]
                nc.vector.tensor_mul(w, pre[:, 4:6], pre[:, 4:6])
                nc.vector.tensor_scalar_mul(
                    w, w, 1.0 / (zscale[l] * zscale[l]))
                nc.vector.tensor_sub(w, pre[:, 6:8], w)           # var
                nc.vector.tensor_mul(w, w, pre[:, 0:2])           # aa*var
                nc.scalar.activation(pre[:, 10:12], w, ACTF.Sqrt, bias=epsap[:])
                nc.vector.reciprocal(w, pre[:, 10:12])            # inv
                nc.vector.tensor_mul(coef[l][:, 0:2], pre[:, 2:4], w)
                nc.vector.tensor_mul(w, coef[l][:, 0:2], pre[:, 4:6])
                nc.vector.tensor_sub(coef[l][:, 2:4], gb[l][:, 2:4], w)
                if l == 0:
                    # thr = -coef_b/coef_s for fill2's DVE-emulated sign
                    # (gamma > 0 so coef_s > 0 and sign(s*z+b)=sign(z-thr))
                    thr = btmp[0][:, 14:16]
                    nc.vector.reciprocal(thr, coef[0][:, 0:2])
                    nc.vector.tensor_mul(thr, thr, coef[0][:, 2:4])
                    nc.vector.tensor_scalar_mul(thr, thr, -1.0)

            # ---------------- one conv pass (shared for conv1/conv2) --------
            def conv_pass(l, act_fill, do_ar=True):
                """act_fill(n, abuf) writes signed fp8 acts into the padded
                [128, NIB*SPP] buffer interior (ring already zero).
                Image n+1's fill is emitted BEFORE image n's matmuls/drains
                so on the in-order ACT/DVE/sync queues the next image's
                loads and signs run during the current matmul stream."""
                act_fill(0, ab2[0])
                if l == 0:
                    # PE HAM warm-up gated on image 0's x arrival (jw1b is
                    # written from the 3rd x chunk): the junk-matmul chain
                    # runs right before conv1's first real matmul, so the
                    # clock-gate is warm with no idle window in between.
                    nc.vector.tensor_scalar(
                        out=jw1b[:], in0=xin_handles[(0, 1, 0)][:, 0:4],
                        scalar1=0.0, scalar2=None, op0=ALU.mult)
                    warmps = pp.tile([128, RBQ], F32, name="cps", tag="cps")
                    NWARM1 = 48
                    for i in range(NWARM1):
                        nc.tensor.matmul(warmps[:, 0:128], jw[:], jw[:],
                                         start=(i == 0),
                                         stop=(i == NWARM1 - 1))
                        if i == 0:
                            nc.tensor.matmul(warmps[:, 0:4], jw[:], jw1b[:],
                                             start=False, stop=False)
                    nc.vector.tensor_reduce(
                        out=btmp[0][:, 13:14], in_=warmps[:, 0:128],
                        axis=AX.X, op=ALU.add)
                for n in range(NPER):
                    abuf = ab2[n % 2]
                    if n + 1 < NPER:
                        act_fill(n + 1, ab2[(n + 1) % 2])
                    if n == NPER - 1 and do_ar:
                        # images 0-2 are done draining by now: AllReduce
                        # their stats, hidden under image 3's matmul stream.
                        # Emitted after image 3's fill so its x DMAs aren't
                        # stuck behind this wait on the sync queue.
                        issue_stats_ar(l, 0)
                    for ob in range(NOB):
                        ps = [pp.tile([128, RBQ], F32, name="cps", tag="cps")
                              for _ in range(RB)]
                        if use_fp8:
                            ab3 = abuf[:].rearrange(
                                "p (two s) -> p two s", two=NIB)
                            for t in range(NTAP):
                                th, tw = t // 3, t % 3
                                base = (ob * NTAP + t) * 2 * 128
                                lhsT = wsign[l][:, base:base + 256].rearrange(
                                    "p (two m) -> p two m", two=2)
                                for rb in range(RB):
                                    r0 = (rb * 8 + th) * HP
                                    if t == 0:
                                        # 466-wide: covers the whole psum
                                        # tile so has_written is uniform
                                        rhs = ab3[:, :, r0:r0 + RBQ]
                                        outap = ps[rb][:, 0:RBQ]
                                    else:
                                        rhs = ab3[:, :, GB + r0:GB + r0 + NMOV]
                                        outap = ps[rb][:, 2 - tw:2 - tw + NMOV]
                                    nc.tensor.matmul(
                                        outap, lhsT, rhs,
                                        start=(t == 0), stop=(t == NTAP - 1),
                                        perf_mode=PM.DoubleRow,
                                    )
                        else:
                            for k in range(NK):
                                t, ib = k // NIB, k % NIB
                                th, tw = t // 3, t % 3
                                kidx = ob * NK + k
                                af = abuf[:, ib * ABW:(ib + 1) * ABW]
                                lhsT = wsign[l][:, kidx * 128:(kidx + 1) * 128]
                                for rb in range(RB):
                                    r0 = (rb * 8 + th) * HP
                                    if no_mm and k > 0:
                                        continue
                                    if k == 0:
                                        rhs = af[:, r0:r0 + RBQ]
                                        outap = ps[rb][:, 0:RBQ]
                                    else:
                                        rhs = af[:, GB + r0:GB + r0 + NMOV]
                                        outap = ps[rb][:, 2 - tw:2 - tw + NMOV]
                                    nc.tensor.matmul(
                                        outap, lhsT, rhs,
                                        start=(k == 0),
                                        stop=(k == NK - 1) or no_mm,
                                    )
                        zs = zstore[l]
                        for rb in range(RB):
                            col = n * RB + rb
                            zsl = zs[:, ((n * NOB + ob) * SP + rb * RBW):
                                      ((n * NOB + ob) * SP + (rb + 1) * RBW)
                                      ].rearrange("p (h w) -> p h w", w=W)
                            qv = ps[rb][:, 2:2 + NMOV].rearrange(
                                "p (h w) -> p h w", w=HP)[:, :, 0:W]
                            # z*zscale store on DVE; accum_out = sum(z*zscale)
                            nc.vector.tensor_scalar(
                                out=zsl, in0=qv,
                                scalar1=zscale[l], scalar2=None, op0=ALU.mult,
                                op1=ALU.add,
                                accum_out=sumc[l][:, ob * 28 + col:
                                                  ob * 28 + col + 1],
                            )
                            # scr = z^2 (dummy out); accum = sum(z^2)
                            scr = pd.tile([128, RBW], F16, name="scr", tag="scr")
                            nc.scalar.activation(
                                out=scr[:].rearrange("p (h w) -> p h w", w=W),
                                in_=qv, func=ACTF.Square,
                                accum_out=sqc[l][:, ob * 28 + col:
                                                 ob * 28 + col + 1],
                            )
                if not do_ar:
                    return
                issue_stats_ar(l, 1)
                fold_bn(l)

            # ---------------- conv1: acts = sign(x) ----------------
            # x arrives in fp32 halves on the sync queue; each half is
            # sign-ed into abuf (exact, from fp32) and copied to the
            # resident fp16 x16 on the gpsimd engine (idle during conv).
            xin_handles = {}

            def fill1(n, abuf):
                for ib in range(NIB):
                    a58 = abuf[:, ib * ABW + GB:ib * ABW + GB + SPP].rearrange(
                        "p (h w) -> p h w", w=HP)
                    xsl = x16[:, (n * NOB + ib) * SP:(n * NOB + ib + 1) * SP]
                    for h in range(2):
                        xin = pd.tile([128, SPH], F32, name="xin", tag="xin",
                                      bufs=3)
                        xin_handles[(n, ib, h)] = xin
                        nc.sync.dma_start(
                            xin[:],
                            x_ap[n, ib * 128:(ib + 1) * 128,
                                 h * SPH:(h + 1) * SPH])
                        xv = xin[:].rearrange("p (h w) -> p h w", w=W)
                        # image 0's fill is on the critical path (runs at
                        # high priority, ahead of the weight-prep signs);
                        # later images' prefetched fills are demoted so the
                        # current image's psum drains win the engine.
                        with tc.high_priority(offset=-3000 if n else None):
                            nc.vector.tensor_copy(
                                xsl[:, h * SPH:(h + 1) * SPH], xin[:])
                            nc.scalar.activation(
                                out=a58[:, 1 + h * 28:1 + (h + 1) * 28,
                                        1:W + 1],
                                in_=xv, func=ACTF.Sign)

            if phase_lim >= 1:
                conv_pass(0, fill1, do_ar=(phase_lim >= 2))

            # park the dummy-AR result so DCE keeps it; the gpsimd queue
            # already waits for the collective, so the park adds no blocking
            # there (on any other queue the scheduler may slot it mid-stream
            # and stall that queue until the dummy AR completes).
            nc.gpsimd.dma_start(btmp[0][:, 12:13], ard_o[:])

            # conv2 weight prep here: its PE transposes overlap the AR1 wait
            weight_prep(1)

            if phase_lim >= 2:
                # PE HAM warm-up: the chain's second link reads jwb (a DVE
                # write gated on the fold output), so on the in-order PE
                # queue the chain runs right after AR1b's fold and keeps
                # the PE busy through fill2 -- conv2 then starts warm.
                nc.vector.tensor_scalar(
                    out=jwb[:], in0=coef[0][:, 0:4],
                    scalar1=0.0, scalar2=None, op0=ALU.mult)
                warm2 = pp.tile([128, RBQ], F32, name="cps", tag="cps")
                NWARM2 = 96
                for i in range(NWARM2):
                    nc.tensor.matmul(warm2[:, 0:128], jw[:], jw[:],
                                     start=(i == 0), stop=(i == NWARM2 - 1))
                    if i == 0:
                        nc.tensor.matmul(warm2[:, 0:4], jw[:], jwb[:],
                                         start=False, stop=False)
                nc.vector.tensor_reduce(
                    out=btmp[1][:, 13:14], in_=warm2[:, 0:128],
                    axis=AX.X, op=ALU.add)

            # ---------------- conv2: acts = sign(s1*z1 + b1) ----------------
            def fill2(n, abuf):
                for ib in range(NIB):
                    a58 = abuf[:, ib * ABW + GB:ib * ABW + GB + SPP].rearrange(
                        "p (h w) -> p h w", w=HP)
                    zv = zstore[0][:, (n * NOB + ib) * SP:
                                   (n * NOB + ib + 1) * SP].rearrange(
                        "p (h w) -> p h w", w=W)
                    with tc.high_priority(offset=-3000 if n else None):
                        if ib == 0:
                            nc.scalar.activation(
                                out=a58[:, 1:H + 1, 1:W + 1], in_=zv,
                                func=ACTF.Sign,
                                scale=coef[0][:, ib:ib + 1],
                                bias=coef[0][:, NOB + ib:NOB + ib + 1],
                            )
                        else:
                            # DVE-emulated sign so the two channel blocks
                            # fill in parallel on ACT and DVE:
                            # (z >= thr)*2 - 1 = sign(s*z + b) for s > 0
                            av = a58[:, 1:H + 1, 1:W + 1]
                            nc.vector.tensor_scalar(
                                out=av, in0=zv,
                                scalar1=btmp[0][:, 14 + ib:15 + ib],
                                scalar2=2.0, op0=ALU.is_ge, op1=ALU.mult)
                            nc.vector.tensor_scalar_add(av, av, -1.0)

            if phase_lim >= 3:
                conv_pass(1, fill2, do_ar=(phase_lim >= 9))

            if phase_lim < 9:
                # debug: dump something touching live tiles into out
                dbg = pd.tile([128, SP], F16, name="dbg", tag="t16", bufs=4)
                if phase_lim >= 1:
                    nc.vector.tensor_copy(dbg[:], zstore[0][:, 0:SP])
                else:
                    nc.vector.tensor_copy(dbg[:], wsign[0][:, 0:SP])
                nc.sync.dma_start(out_ap[0, 0:128, :], dbg[:])

            # ---------------- finalize: out = s2*z2' + b2 + x ----------------
            # Per block: per-channel affine (alternating ACT / DVE so neither
            # engine is the serial bottleneck), DVE residual add from the
            # resident fp16 x, fp16 store rotated over 3 DMA queues.
            for n in range(NPER if phase_lim >= 9 else 0):
                for ob in range(NOB):
                    k = n * NOB + ob
                    zsl = zstore[1][:, (n * NOB + ob) * SP:
                                    (n * NOB + ob + 1) * SP]
                    xsl = x16[:, (n * NOB + ob) * SP:(n * NOB + ob + 1) * SP]
                    t16 = pd.tile([128, SP], F16, name="t16", tag="t16",
                                  bufs=4)
                    if k % 2 == 0 or k == 7:
                        nc.scalar.activation(
                            out=t16[:], in_=zsl, func=ACTF.Identity,
                            scale=coef[1][:, ob:ob + 1],
                            bias=coef[1][:, NOB + ob:NOB + ob + 1],
                        )
                    else:
                        nc.vector.tensor_scalar(
                            out=t16[:], in0=zsl,
                            scalar1=coef[1][:, ob:ob + 1],
                            scalar2=coef[1][:, NOB + ob:NOB + ob + 1],
                            op0=ALU.mult, op1=ALU.add,
                        )
                    nc.vector.tensor_add(t16[:], t16[:], xsl)
                    dma_eng = (nc.sync, nc.scalar)[k % 2]
                    dma_eng.dma_start(
                        out_ap[n, ob * 128:(ob + 1) * 128, :], t16[:])

    nc.compile()
    return nc


def _get_nc(num_devices=NCORES):
    if num_devices not in _nc_cache:
        _nc_cache[num_devices] = build_nc(num_devices)
    return _nc_cache[num_devices]


def kernel(**inputs):
    from concourse.bass_utils import run_bass_kernel_spmd

    nc = _get_nc(NCORES)
    x = np.ascontiguousarray(np.asarray(inputs["x"], dtype=np.float32))
    shared = {
        k: np.ascontiguousarray(np.asarray(inputs[k], dtype=np.float32))
        for k in ("w1", "gamma1", "beta1", "w2", "gamma2", "beta2")
    }
    in_maps = [
        {"x": x[c * NPER:(c + 1) * NPER], **shared} for c in range(NCORES)
    ]
    res = run_bass_kernel_spmd(nc, in_maps, core_ids=list(range(NCORES)))
    out = np.concatenate([np.asarray(r["out"]) for r in res.results], axis=0)
    return out.astype(np.float32)
